# revision 5
# baseline (speedup 1.0000x reference)
"""Trainium2 Bass kernel: attention-LSTM decoder (teacher-forced), 8 NeuronCores.

Strategy: the LSTM recurrence is the only sequential part. Everything else
(embedding @ W_ih, attention, fc1, fc2) is batched over all T steps.
Phases 0-3 are replicated on all 8 cores; fc2 (the dominant matmul,
[B*T,H] @ [H,V]) is sharded over the vocab dim (V/8 = 4000 per core).
No collectives: the host concatenates the 8 logits shards.

Layouts (device):
  gates column order is rearranged (host-side) so that each PSUM pass holds
  gate pairs interleaved per 256-wide h-window:
    pass0: [i | g] per window, pass1: [f | o] per window.
  PSUM partition p = 32*j + b  (j = h-window 0..3, b = batch 0..31)
  -> LSTM elementwise runs on all 128 partitions.
  h is transposed back each step (PE transpose) into HsT[hi, ko, b, t].
"""

import numpy as np
import ml_dtypes

BF16 = ml_dtypes.bfloat16

V, E, H, B, T, S = 32000, 512, 1024, 32, 64, 64
NCORES = 8
VS = V // NCORES  # 4000 vocab cols per core
BT = B * T  # 2048


def _col_order():
    """Column permutation of the 4H gate dim used by W_ih/W_hh/bias on device."""
    order = []
    for p2 in range(2):
        ga = 0 if p2 == 0 else 1024      # i or f
        gb = 2048 if p2 == 0 else 3072   # g or o
        for j in range(4):
            order.extend(range(ga + j * 256, ga + (j + 1) * 256))
            order.extend(range(gb + j * 256, gb + (j + 1) * 256))
    return np.asarray(order, dtype=np.int64)


_NC = None


def _phase0(nc, tc, dt, AF, xeT, wih, biasg, Pd):
    """P[t] = xe[t] @ W_ih^T + bias, batched over all t, -> DRAM (bf16)."""
    with tc.tile_pool(name="ph0", bufs=1) as p0, \
         tc.tile_pool(name="ph0ps", bufs=4, space="PSUM") as ps0, \
         tc.tile_pool(name="ph0st", bufs=4) as st0:
        xeT_sb = p0.tile([128, T, 4, 32], dt.bfloat16, tag="xeT")
        nc.sync.dma_start(xeT_sb[:], xeT[:])
        wih_sb = p0.tile([128, 4, 4096], dt.bfloat16, tag="wih")
        nc.sync.dma_start(wih_sb[:], wih[:])
        bias_sb = p0.tile([128, 2, 512], dt.float32, tag="biasg")
        nc.sync.dma_start(bias_sb[:], biasg[:])

        for t in range(T):
            for p2 in range(2):
                ps = ps0.tile([128, 512], dt.float32, tag="ps")
                for ko in range(4):
                    for j in range(4):
                        nc.tensor.matmul(
                            ps[32 * j:32 * (j + 1), :],
                            lhsT=xeT_sb[:, t, ko, :],
                            rhs=wih_sb[:, ko, (p2 * 4 + j) * 512:(p2 * 4 + j + 1) * 512],
                            start=(ko == 0), stop=(ko == 3),
                            skip_group_check=True,
                            tile_position=(0, 32 * j),
                        )
                sb = st0.tile([128, 512], dt.bfloat16, tag="pstage")
                nc.vector.tensor_add(sb[:], ps[:], bias_sb[:, p2, :])
                nc.sync.dma_start(Pd[t, p2], sb[:])


def _phase1(nc, tc, dt, AF, h0T, c0, Pd, whh_sb, ident_sb, HsT):
    """Sequential LSTM recurrence; writes HsT[:, :, :, t] per step."""
    with tc.tile_pool(name="ph1", bufs=1) as p1, \
         tc.tile_pool(name="ph1ps", bufs=4, space="PSUM") as ps1, \
         tc.tile_pool(name="ph1tr", bufs=2, space="PSUM") as ps1t, \
         tc.tile_pool(name="ph1pt", bufs=6) as ptp, \
         tc.tile_pool(name="ph1st", bufs=3) as st1:
        h0T_sb = p1.tile([128, 8, 32], dt.bfloat16, tag="h0T")
        nc.sync.dma_start(h0T_sb[:], h0T[:])
        c_sb = p1.tile([128, 256], dt.float32, tag="c")
        nc.sync.dma_start(c_sb[:], c0[:])

        for t in range(T):
            def hT(ko, _t=t):
                if _t == 0:
                    return h0T_sb[:, ko, :]
                return HsT[:, ko, :, _t - 1]

            pspass = []
            for p2 in range(2):
                pt_sb = ptp.tile([128, 512], dt.bfloat16, tag="pt")
                nc.sync.dma_start(pt_sb[:], Pd[t, p2])
                ps = ps1.tile([128, 512], dt.float32, tag="gates")
                for ko in range(8):
                    for j in range(4):
                        nc.tensor.matmul(
                            ps[32 * j:32 * (j + 1), :],
                            lhsT=hT(ko),
                            rhs=whh_sb[:, ko, (p2 * 4 + j) * 512:(p2 * 4 + j + 1) * 512],
                            start=(ko == 0), stop=False,
                            skip_group_check=True,
                            tile_position=(0, 32 * j),
                        )
                # += P_t  (identity-matmul add, exact)
                nc.tensor.matmul(ps[:, :], lhsT=ident_sb[:], rhs=pt_sb[:],
                                 start=False, stop=True, skip_group_check=True)
                pspass.append(ps)

            ig_sb = st1.tile([128, 512], dt.float32, tag="ig")
            nc.scalar.activation(ig_sb[:, 0:256], pspass[0][:, 0:256], AF.Sigmoid)
            nc.scalar.activation(ig_sb[:, 256:512], pspass[0][:, 256:512], AF.Tanh)
            fo_sb = st1.tile([128, 512], dt.float32, tag="fo")
            nc.scalar.activation(fo_sb[:], pspass[1][:], AF.Sigmoid)

            ig2 = st1.tile([128, 256], dt.float32, tag="ig2")
            nc.vector.tensor_mul(ig2[:], ig_sb[:, 0:256], ig_sb[:, 256:512])
            fc_ = st1.tile([128, 256], dt.float32, tag="fc")
            nc.vector.tensor_mul(fc_[:], fo_sb[:, 0:256], c_sb[:])
            nc.vector.tensor_add(c_sb[:], fc_[:], ig2[:])
            thc = st1.tile([128, 256], dt.float32, tag="thc")
            nc.scalar.activation(thc[:], c_sb[:], AF.Tanh)
            h_sb = st1.tile([128, 256], dt.bfloat16, tag="h")
            nc.vector.tensor_mul(h_sb[:], fo_sb[:, 256:512], thc[:])

            # transpose h back to [hi, (ko,b)] and store into HsT[:, :, :, t]
            for half in range(2):
                trp = ps1t.tile([128, 128], dt.bfloat16, tag="tr")
                nc.tensor.transpose(trp[:], h_sb[:, half * 128:(half + 1) * 128],
                                    ident_sb[:])
                nc.vector.tensor_copy(
                    HsT[:, half:8:2, :, t],
                    trp[:].rearrange("p (j b) -> p j b", j=4),
                )


def _phase2(nc, tc, dt, AF, mybir, encT, encsp, HsT, ctxT, ident_sb):
    """Attention: scores -> softmax -> ctxT, batched over (b, t)."""
    with tc.tile_pool(name="att", bufs=1) as ap_, \
         tc.tile_pool(name="attps", bufs=2, space="PSUM") as aps, \
         tc.tile_pool(name="attpsT", bufs=2, space="PSUM") as apsT, \
         tc.tile_pool(name="attpsC", bufs=2, space="PSUM") as apsC, \
         tc.tile_pool(name="attst", bufs=3) as ast:
        encT_sb = ap_.tile([128, 8, 32, 64], dt.bfloat16, tag="encT")
        nc.sync.dma_start(encT_sb[:], encT[:])
        encsp_sb = ap_.tile([64, 32, 8, 128], dt.bfloat16, tag="encsp")
        nc.sync.dma_start(encsp_sb[:], encsp[:])

        for bg in range(4):  # 8 batch elements per group
            ps_sc = aps.tile([64, 512], dt.float32, tag="sc")
            for bi in range(8):
                b = bg * 8 + bi
                for ko in range(8):
                    nc.tensor.matmul(
                        ps_sc[:, bi * 64:(bi + 1) * 64],
                        lhsT=HsT[:, ko, b, :],
                        rhs=encT_sb[:, ko, b, :],
                        start=(ko == 0), stop=(ko == 7),
                        skip_group_check=True,
                    )
            # softmax over s (innermost 64)
            mx = ast.tile([64, 8], dt.float32, tag="mx")
            nc.vector.reduce_max(mx[:], ps_sc[:].rearrange("p (b s) -> p b s", s=64),
                                 axis=mybir.AxisListType.X)
            sc2 = ast.tile([64, 8, 64], dt.float32, tag="sc2")
            nc.vector.tensor_sub(sc2[:], ps_sc[:].rearrange("p (b s) -> p b s", s=64),
                                 mx[:, :, None].to_broadcast((64, 8, 64)))
            nc.scalar.activation(sc2[:], sc2[:], AF.Exp)
            sm = ast.tile([64, 8], dt.float32, tag="sm")
            nc.vector.reduce_sum(sm[:], sc2[:], axis=mybir.AxisListType.X)
            nc.vector.reciprocal(sm[:], sm[:])
            a_sb = ast.tile([64, 8, 64], dt.bfloat16, tag="a")
            nc.vector.tensor_mul(a_sb[:], sc2[:],
                                 sm[:, :, None].to_broadcast((64, 8, 64)))

            for bi in range(8):
                b = bg * 8 + bi
                psT = apsT.tile([64, 64], dt.bfloat16, tag="aT")
                nc.tensor.transpose(psT[:], a_sb[:, bi, :], ident_sb[0:64, 0:64])
                aT_sb = ast.tile([64, 64], dt.bfloat16, tag="aTs")
                nc.vector.tensor_copy(aT_sb[:], psT[:])
                ps_ctx = apsC.tile([128, 512], dt.float32, tag="ctx")
                for ho in range(8):
                    nc.tensor.matmul(
                        ps_ctx[:, ho * 64:(ho + 1) * 64],
                        lhsT=encsp_sb[:, b, ho, :],
                        rhs=aT_sb[:],
                        start=True, stop=True,
                        skip_group_check=True,
                    )
                nc.vector.tensor_copy(
                    ctxT[:, :, b, :],
                    ps_ctx[:].rearrange("p (ho t) -> p ho t", t=64),
                )


def _phase3(nc, tc, dt, AF, fc1w, fc1b, HsT, ctxT, ZT):
    """z = tanh([h; ctx] @ fc1_W^T + b1), output ZT[hi, ho, (b,t)]."""
    with tc.tile_pool(name="f1", bufs=1) as f1p, \
         tc.tile_pool(name="f1ps", bufs=4, space="PSUM") as f1ps:
        fc1w_sb = f1p.tile([128, 16, 1024], dt.bfloat16, tag="fc1w")
        nc.sync.dma_start(fc1w_sb[:], fc1w[:])
        fc1b_sb = f1p.tile([128, 8], dt.float32, tag="fc1b")
        nc.sync.dma_start(fc1b_sb[:], fc1b[:])

        for mo in range(8):
            for nt in range(4):
                ps = f1ps.tile([128, 512], dt.float32, tag="ps")
                for ko in range(16):
                    if ko < 8:
                        rhs = HsT[:, ko, nt * 8:(nt + 1) * 8, :]
                    else:
                        rhs = ctxT[:, ko - 8, nt * 8:(nt + 1) * 8, :]
                    nc.tensor.matmul(
                        ps[:],
                        lhsT=fc1w_sb[:, ko, mo * 128:(mo + 1) * 128],
                        rhs=rhs,
                        start=(ko == 0), stop=(ko == 15),
                        skip_group_check=True,
                    )
                nc.scalar.activation(ZT[:, mo, nt * 512:(nt + 1) * 512], ps[:],
                                     AF.Tanh, bias=fc1b_sb[:, mo:mo + 1])


def _phase4(nc, tc, dt, AF, fc2w, fc2b, out, ZT):
    """logits slice = Z @ fc2_W_slice^T + b2 -> out DRAM."""
    with tc.tile_pool(name="f2", bufs=1) as f2p, \
         tc.tile_pool(name="f2w", bufs=2) as f2wp, \
         tc.tile_pool(name="f2ps", bufs=4, space="PSUM") as f2ps, \
         tc.tile_pool(name="f2st", bufs=4) as f2st:
        fc2b_sb = f2p.tile([128, VS], dt.float32, tag="fc2b")
        nc.sync.dma_start(fc2b_sb[:], fc2b[:])

        for vo in range(8):
            wv = f2wp.tile([128, 8, 500], dt.bfloat16, tag="wv")
            nc.sync.dma_start(wv[:], fc2w[:, :, vo * 500:(vo + 1) * 500])
            for mo in range(16):
                ps = f2ps.tile([128, 500], dt.float32, tag="ps")
                for ko in range(8):
                    nc.tensor.matmul(
                        ps[:],
                        lhsT=ZT[:, ko, mo * 128:(mo + 1) * 128],
                        rhs=wv[:, ko, :],
                        start=(ko == 0), stop=(ko == 7),
                        skip_group_check=True,
                    )
                ob = f2st.tile([128, 500], dt.float32, tag="ob")
                nc.vector.tensor_add(ob[:], ps[:], fc2b_sb[:, vo * 500:(vo + 1) * 500])
                nc.sync.dma_start(out[mo * 128:(mo + 1) * 128, vo * 500:(vo + 1) * 500],
                                  ob[:])


def _build():
    """Build the Bass graph (single NeuronCore program, SPMD across 8)."""
    import concourse.mybir as mybir
    from concourse import bacc
    import concourse.tile as tile

    dt = mybir.dt
    AF = mybir.ActivationFunctionType

    nc = bacc.Bacc(None, target_bir_lowering=False)

    def inp(name, shape, dtp):
        return nc.declare_dram_parameter(name, list(shape), dtp, isOutput=False)

    xeT = inp("xeT", (128, T, 4, 32), dt.bfloat16)       # emb[inputs] transposed
    wih = inp("wih", (128, 4, 4096), dt.bfloat16)        # W_ih^T, arranged cols
    whh = inp("whh", (128, 8, 4096), dt.bfloat16)        # W_hh^T, arranged cols
    biasg = inp("biasg", (128, 2, 512), dt.float32)      # (b_ih+b_hh) arranged
    ident = inp("ident", (128, 128), dt.bfloat16)
    encT = inp("encT", (128, 8, 32, 64), dt.bfloat16)    # enc[h,b,s] h-part
    encsp = inp("encsp", (64, 32, 8, 128), dt.bfloat16)  # enc[s,b,ho,hi] s-part
    h0T = inp("h0T", (128, 8, 32), dt.bfloat16)
    c0 = inp("c0", (128, 256), dt.float32)               # cell, (j,b) layout
    fc1w = inp("fc1w", (128, 16, 1024), dt.bfloat16)     # fc1_W^T
    fc1b = inp("fc1b", (128, 8), dt.float32)
    fc2w = inp("fc2w", (128, 8, VS), dt.bfloat16)        # per-core V slice
    fc2b = inp("fc2b", (128, VS), dt.float32)            # bias bcast over rows
    out = nc.declare_dram_parameter("out", [BT, VS], dt.float32, isOutput=True)

    Pd = nc.dram_tensor("Pscratch", [T, 2, 128, 512], dt.bfloat16)

    with tile.TileContext(nc) as tc:
        with tc.tile_pool(name="persist", bufs=1) as pp, \
             tc.tile_pool(name="hstp", bufs=1) as hstp:
            ident_sb = pp.tile([128, 128], dt.bfloat16, tag="ident")
            nc.sync.dma_start(ident_sb[:], ident[:])
            HsT = hstp.tile([128, 8, 32, 64], dt.bfloat16, tag="HsT")

            with tc.tile_pool(name="whhp", bufs=1) as whp:
                whh_sb = whp.tile([128, 8, 4096], dt.bfloat16, tag="whh")
                nc.sync.dma_start(whh_sb[:], whh[:])
                _phase0(nc, tc, dt, AF, xeT, wih, biasg, Pd)
                _phase1(nc, tc, dt, AF, h0T, c0, Pd, whh_sb, ident_sb, HsT)

            with tc.tile_pool(name="ctxp", bufs=1) as ctp:
                ctxT = ctp.tile([128, 8, 32, 64], dt.bfloat16, tag="ctxT")
                _phase2(nc, tc, dt, AF, mybir, encT, encsp, HsT, ctxT, ident_sb)
                with tc.tile_pool(name="ztp", bufs=1) as ztp:
                    ZT = ztp.tile([128, 8, BT], dt.bfloat16, tag="ZT")
                    _phase3(nc, tc, dt, AF, fc1w, fc1b, HsT, ctxT, ZT)
                    _phase4(nc, tc, dt, AF, fc2w, fc2b, out, ZT)

    nc.compile()
    return nc


def _get_nc():
    global _NC
    if _NC is None:
        _NC = _build()
    return _NC


def _prep_inputs(inputs, hiddens, hidden, cell, emb, W_ih, b_ih, W_hh, b_hh,
                 fc1_W, fc1_b, fc2_W, fc2_b):
    """Host-side layout prep (gather / transpose / cast only)."""
    order = _col_order()
    f32 = np.float32

    inds = np.asarray(inputs).astype(np.int64)
    xe = np.asarray(emb, f32)[inds]                      # [B, T, E]
    xeT = np.ascontiguousarray(
        xe.reshape(B, T, 4, 128).transpose(3, 1, 2, 0)).astype(BF16)

    wih_a = np.ascontiguousarray(
        np.asarray(W_ih, f32).T[:, order].reshape(4, 128, 4096)
        .transpose(1, 0, 2)).astype(BF16)
    whh_a = np.ascontiguousarray(
        np.asarray(W_hh, f32).T[:, order].reshape(8, 128, 4096)
        .transpose(1, 0, 2)).astype(BF16)

    bias_vec = (np.asarray(b_ih, f32) + np.asarray(b_hh, f32))[order]
    # bias[32j+b, p2, n] = ordered[p2, j, n]
    biasg = np.ascontiguousarray(
        np.repeat(bias_vec.reshape(2, 4, 512).transpose(1, 0, 2), 32, axis=0))

    ident = np.eye(128, dtype=f32).astype(BF16)

    hid = np.asarray(hiddens, f32)                       # [S, B, H]
    encT = np.ascontiguousarray(
        hid.reshape(S, B, 8, 128).transpose(3, 2, 1, 0)).astype(BF16)
    encsp = np.ascontiguousarray(hid.reshape(S, B, 8, 128)).astype(BF16)

    h0T = np.ascontiguousarray(
        np.asarray(hidden, f32).reshape(B, 8, 128).transpose(2, 1, 0)).astype(BF16)
    c0a = np.ascontiguousarray(
        np.asarray(cell, f32).reshape(B, 4, 256).transpose(1, 0, 2).reshape(128, 256))

    fc1w_a = np.ascontiguousarray(
        np.asarray(fc1_W, f32).T.reshape(16, 128, 1024).transpose(1, 0, 2)).astype(BF16)
    fc1b_a = np.ascontiguousarray(np.asarray(fc1_b, f32).reshape(8, 128).T)

    common = dict(xeT=xeT, wih=wih_a, whh=whh_a, biasg=biasg, ident=ident,
                  encT=encT, encsp=encsp, h0T=h0T, c0=c0a,
                  fc1w=fc1w_a, fc1b=fc1b_a)

    fc2_W = np.asarray(fc2_W, f32)
    fc2_b = np.asarray(fc2_b, f32)
    in_maps = []
    for r in range(NCORES):
        sl = slice(r * VS, (r + 1) * VS)
        fc2w_r = np.ascontiguousarray(
            fc2_W[sl].T.reshape(8, 128, VS).transpose(1, 0, 2)).astype(BF16)
        fc2b_r = np.ascontiguousarray(
            np.broadcast_to(fc2_b[sl], (128, VS)))
        in_maps.append({**common, "fc2w": fc2w_r, "fc2b": fc2b_r})
    return in_maps


def kernel(inputs, hiddens, hidden, cell, emb, W_ih, b_ih, W_hh, b_hh,
           fc1_W, fc1_b, fc2_W, fc2_b, generate_len=None, _trace=False,
           _tmpdir=None):
    from concourse.bass_utils import run_bass_kernel_spmd

    in_maps = _prep_inputs(inputs, hiddens, hidden, cell, emb, W_ih, b_ih,
                           W_hh, b_hh, fc1_W, fc1_b, fc2_W, fc2_b)
    nc = _get_nc()
    res = run_bass_kernel_spmd(nc, in_maps, list(range(NCORES)),
                               trace=_trace, tmpdir=_tmpdir)
    shards = [np.asarray(res.results[r]["out"], np.float32) for r in range(NCORES)]
    full = np.concatenate(shards, axis=1)           # [B*T, V], rows b*T+t
    out = full.reshape(B, T, V)
    if _trace:
        return out, res
    return out
